# revision 6
# baseline (speedup 1.0000x reference)
"""ActorCritic segment-reduce kernel for 8 TRN2 NeuronCores.

Strategy (data-parallel over graph batch B=512 -> 64 graphs/core):
  - Critic is evaluated ONLY on gathered rows (64 sel + 2048 next-node rows
    per core) via indirect DMA, instead of all 102400 rows (the headroom).
  - Rows are PE-transposed so the contract dim lands on partitions; MLPs run
    in float32r (single-pass fp32 matmul, ~tf32 precision).
  - Critic layer 2 (H @ Wc2) is one fused DVE scalar_tensor_tensor:
    out=(relu(psum))*Wc2rep with accum_out giving the row dot product.
  - Segment max over K=32 next-nodes: host orders gather rows k-major so the
    seg-max becomes a free-dim reduce over 16 stacked columns + one
    cross-partition-half max.
  - Actor (64 rows x A=4096): f32r matmuls; masked softmax epilogue uses the
    scalar engine Exp with accum_out for sum-exp; logp[b, xfers[b]] is an
    indirect 4B-gather from a DRAM round-trip of the masked logits.
  - Weights replicated; biases applied via rank-1 (ones x bias) matmuls or
    per-partition activation bias. No cross-core communication; host
    concatenates the 4 per-core [64] outputs and does the O(B) scalar math
    (log-sum-exp combine is fully per-graph on device; host only concats and
    takes the entropy mean).
"""
import numpy as np

import concourse.bass as bass
import concourse.mybir as mybir
import concourse.tile as tile
from concourse import bacc
from concourse.bass import IndirectOffsetOnAxis
from concourse.bass_utils import run_bass_kernel_spmd
from concourse.masks import make_identity

F32 = mybir.dt.float32
F32R = mybir.dt.float32r
I32 = mybir.dt.int32
U8 = mybir.dt.uint8
AF = mybir.ActivationFunctionType
OP = mybir.AluOpType

B, N, D = 512, 200, 256
HC, HA, A = 512, 512, 4096
K = 32
NCORES = 8
BL = B // NCORES            # 64 graphs per core
RN = BL * K                 # 2048 gathered next rows per core
NT = RN // 128              # 16 gather tiles
P = 128
ACH = A // 512              # 8 actor column chunks of 512

_COMPILED = None


def _build():
    nc = bacc.Bacc("TRN2", target_bir_lowering=False, debug=False,
                   num_devices=NCORES)

    ge = nc.dram_tensor("ge", [BL * N, D], F32R, kind="ExternalInput")
    nge = nc.dram_tensor("nge", [BL * N, D], F32R, kind="ExternalInput")
    wc1 = nc.dram_tensor("wc1", [D, HC], F32R, kind="ExternalInput")
    bc1 = nc.dram_tensor("bc1", [HC], F32R, kind="ExternalInput")
    wc2 = nc.dram_tensor("wc2", [HC], F32, kind="ExternalInput")
    bc2 = nc.dram_tensor("bc2", [1], F32, kind="ExternalInput")
    wa1 = nc.dram_tensor("wa1", [D, HA], F32R, kind="ExternalInput")
    ba1 = nc.dram_tensor("ba1", [HA], F32, kind="ExternalInput")
    wa2 = nc.dram_tensor("wa2", [HA, A], F32R, kind="ExternalInput")
    ba2 = nc.dram_tensor("ba2", [A], F32R, kind="ExternalInput")
    masks = nc.dram_tensor("masks", [BL, A], U8, kind="ExternalInput")
    term = nc.dram_tensor("term", [BL], U8, kind="ExternalInput")
    sel_idx = nc.dram_tensor("sel_idx", [BL], I32, kind="ExternalInput")
    next_idx = nc.dram_tensor("next_idx", [RN], I32, kind="ExternalInput")
    xf_idx = nc.dram_tensor("xf_idx", [BL], I32, kind="ExternalInput")

    values_o = nc.dram_tensor("values_o", [BL], F32, kind="ExternalOutput")
    nextv_o = nc.dram_tensor("nextv_o", [BL], F32, kind="ExternalOutput")
    xlp_o = nc.dram_tensor("xlp_o", [BL], F32, kind="ExternalOutput")
    ent_o = nc.dram_tensor("ent_o", [BL], F32, kind="ExternalOutput")

    ml_dram = nc.dram_tensor("ml_dram", [BL * A, 1], F32)  # internal

    with tile.TileContext(nc) as tc:
        with (
            tc.tile_pool(name="const", bufs=1) as const,
            tc.tile_pool(name="work", bufs=3) as work,
            tc.tile_pool(name="junk", bufs=2) as junkp,
            tc.tile_pool(name="small", bufs=4) as small,
            tc.tile_pool(name="pt", bufs=2, space="PSUM") as pt,
            tc.tile_pool(name="pm", bufs=3, space="PSUM") as pm,
            tc.tile_pool(name="pa", bufs=2, space="PSUM") as pa,
        ):
            # ---------------- constants / weights to SBUF ----------------
            ident_f = const.tile([P, P], F32)
            make_identity(nc, ident_f[:])
            ident = const.tile([P, P], F32R)
            nc.vector.tensor_copy(out=ident[:], in_=ident_f[:])
            ones_f = const.tile([1, P], F32)
            nc.vector.memset(ones_f[:], 1.0)
            ones = const.tile([1, P], F32R)
            nc.vector.tensor_copy(out=ones[:], in_=ones_f[:])

            wc1_t = const.tile([P, 2, HC], F32R)
            nc.sync.dma_start(out=wc1_t[:, 0, :], in_=wc1[0:128, :])
            nc.sync.dma_start(out=wc1_t[:, 1, :], in_=wc1[128:256, :])
            wa1_t = const.tile([P, 2, HA], F32R)
            nc.sync.dma_start(out=wa1_t[:, 0, :], in_=wa1[0:128, :])
            nc.sync.dma_start(out=wa1_t[:, 1, :], in_=wa1[128:256, :])
            wa2_t = const.tile([P, 4, A], F32R)
            for j in range(4):
                nc.sync.dma_start(out=wa2_t[:, j, :], in_=wa2[j * 128:(j + 1) * 128, :])

            bc1_r = const.tile([1, HC], F32R)
            nc.sync.dma_start(out=bc1_r[:], in_=bc1[None, :])
            ba2_r = const.tile([1, A], F32R)
            nc.sync.dma_start(out=ba2_r[:], in_=ba2[None, :])

            wc2_row = const.tile([1, HC], F32)
            nc.sync.dma_start(out=wc2_row[:], in_=wc2[None, :])
            wc2_rep = const.tile([P, HC], F32)
            nc.gpsimd.partition_broadcast(wc2_rep[:], wc2_row[:], channels=P)

            bc2_row = const.tile([1, 1], F32)
            nc.sync.dma_start(out=bc2_row[:], in_=bc2[None, :])
            bc2_rep = const.tile([P, 1], F32)
            nc.gpsimd.partition_broadcast(bc2_rep[:], bc2_row[:], channels=P)

            ba1_s = const.tile([P, 4], F32)
            nc.sync.dma_start(out=ba1_s[:], in_=ba1.ap().rearrange("(j p) -> p j", p=P))

            masks_sb = const.tile([BL, A], U8)
            nc.sync.dma_start(out=masks_sb[:], in_=masks[:, :])
            term_sb = const.tile([BL, 1], U8)
            nc.sync.dma_start(out=term_sb[:], in_=term[:, None])

            selidx_sb = const.tile([BL, 1], I32)
            nc.sync.dma_start(out=selidx_sb[:], in_=sel_idx[:, None])
            nidx_sb = const.tile([P, NT], I32)
            nc.sync.dma_start(out=nidx_sb[:], in_=next_idx.ap().rearrange("(t p) -> p t", p=P))
            xf_sb = const.tile([BL, 1], I32)
            nc.sync.dma_start(out=xf_sb[:], in_=xf_idx[:, None])

            # =============== critic on gathered sel rows (values) ========
            xsel = work.tile([BL, D], F32R, tag="gx")
            nc.gpsimd.indirect_dma_start(
                out=xsel[:], out_offset=None, in_=ge[:, :],
                in_offset=IndirectOffsetOnAxis(ap=selidx_sb[:, :1], axis=0))
            selT = work.tile([P, 2, BL], F32R, tag="xt")
            for c in range(2):
                tp = pt.tile([P, BL], F32R)
                nc.tensor.transpose(out=tp[:], in_=xsel[:, c * P:(c + 1) * P],
                                    identity=ident[:BL, :BL])
                nc.vector.tensor_copy(out=selT[:, c, :], in_=tp[:])

            ps = pm.tile([BL, HC], F32, tag="mm")
            nc.tensor.matmul(out=ps[:], lhsT=selT[:, 0, :], rhs=wc1_t[:, 0, :],
                             start=True, stop=False)
            nc.tensor.matmul(out=ps[:], lhsT=selT[:, 1, :], rhs=wc1_t[:, 1, :],
                             start=False, stop=False)
            nc.tensor.matmul(out=ps[:], lhsT=ones[:, :BL], rhs=bc1_r[:],
                             start=False, stop=True)
            jt = junkp.tile([P, HC], F32, tag="junk")
            vsel = small.tile([BL, 1], F32)
            nc.vector.scalar_tensor_tensor(
                out=jt[:BL, :], in0=ps[:], scalar=0.0, in1=wc2_rep[:BL, :],
                op0=OP.max, op1=OP.mult, accum_out=vsel[:])
            # values = vsel + bc2
            vals = small.tile([BL, 1], F32)
            nc.vector.tensor_scalar_add(vals[:], vsel[:], bc2_rep[:BL, :1])
            nc.sync.dma_start(out=values_o[:, None], in_=vals[:])

            # =============== critic on gathered next rows (seg-max) ======
            v_all = small.tile([P, NT], F32)
            for t in range(NT):
                xn = work.tile([P, D], F32R, tag="gx")
                nc.gpsimd.indirect_dma_start(
                    out=xn[:], out_offset=None, in_=nge[:, :],
                    in_offset=IndirectOffsetOnAxis(ap=nidx_sb[:, t:t + 1], axis=0))
                xT = work.tile([P, 2, P], F32R, tag="xt")
                for c in range(2):
                    tp = pt.tile([P, P], F32R)
                    nc.tensor.transpose(out=tp[:], in_=xn[:, c * P:(c + 1) * P],
                                        identity=ident[:])
                    nc.vector.tensor_copy(out=xT[:, c, :], in_=tp[:])
                pn = pm.tile([P, HC], F32, tag="mm")
                nc.tensor.matmul(out=pn[:], lhsT=xT[:, 0, :], rhs=wc1_t[:, 0, :],
                                 start=True, stop=False)
                nc.tensor.matmul(out=pn[:], lhsT=xT[:, 1, :], rhs=wc1_t[:, 1, :],
                                 start=False, stop=False)
                nc.tensor.matmul(out=pn[:], lhsT=ones[:], rhs=bc1_r[:],
                                 start=False, stop=True)
                jn = junkp.tile([P, HC], F32, tag="junk")
                nc.vector.scalar_tensor_tensor(
                    out=jn[:], in0=pn[:], scalar=0.0, in1=wc2_rep[:],
                    op0=OP.max, op1=OP.mult, accum_out=v_all[:, t:t + 1])

            vmax = small.tile([P, 1], F32)
            nc.vector.tensor_reduce(out=vmax[:], in_=v_all[:],
                                    axis=mybir.AxisListType.X, op=OP.max)
            # rows are k-major (r = k*64 + b): partitions p and p+64 hold
            # (even k, b=p) and (odd k, b=p-64); combine the halves.
            vhi = small.tile([BL, 1], F32)
            nc.sync.dma_start(out=vhi[:], in_=vmax[BL:P, :])
            nv1 = small.tile([BL, 1], F32)
            nc.vector.tensor_tensor(out=nv1[:], in0=vmax[0:BL, :],
                                    in1=vhi[:], op=OP.max)
            # (1 - terminal) factor
            tf = small.tile([BL, 1], F32)
            nc.scalar.activation(out=tf[:], in_=term_sb[:], func=AF.Copy,
                                 scale=-1.0, bias=1.0)
            nv = small.tile([BL, 1], F32)
            nc.vector.scalar_tensor_tensor(
                out=nv[:], in0=nv1[:], scalar=bc2_rep[:BL, :1], in1=tf[:],
                op0=OP.add, op1=OP.mult)
            nc.sync.dma_start(out=nextv_o[:, None], in_=nv[:])

            # ======================= actor ===============================
            ha = const.tile([P, 4, BL], F32R)  # H_a laid out [h, b]
            for j in range(4):
                pl1 = pa.tile([P, BL], F32)
                nc.tensor.matmul(out=pl1[:], lhsT=wa1_t[:, 0, j * 128:(j + 1) * 128],
                                 rhs=selT[:, 0, :], start=True, stop=False)
                nc.tensor.matmul(out=pl1[:], lhsT=wa1_t[:, 1, j * 128:(j + 1) * 128],
                                 rhs=selT[:, 1, :], start=False, stop=True)
                nc.scalar.activation(out=ha[:, j, :], in_=pl1[:], func=AF.Relu,
                                     bias=ba1_s[:, j:j + 1])

            ml_all = const.tile([BL, A], F32)
            rm_all = small.tile([BL, ACH], F32)
            ml_view = ml_dram.ap().rearrange("(b a) one -> b (a one)", b=BL)
            for j in range(ACH):
                asl = slice(j * 512, (j + 1) * 512)
                pl2 = pm.tile([BL, 512], F32, tag="mm")
                for h in range(4):
                    nc.tensor.matmul(out=pl2[:], lhsT=ha[:, h, :],
                                     rhs=wa2_t[:, h, asl],
                                     start=(h == 0), stop=False)
                nc.tensor.matmul(out=pl2[:], lhsT=ones[:, :BL], rhs=ba2_r[:, asl],
                                 start=False, stop=True)
                # mask term: 1e10*mask - 1e10  (0 where legal, -1e10 where not)
                mterm = junkp.tile([BL, 512], F32, tag="mterm")
                nc.scalar.activation(out=mterm[:], in_=masks_sb[:, asl],
                                     func=AF.Copy, scale=1e10, bias=-1e10)
                nc.vector.tensor_tensor(out=ml_all[:, asl], in0=pl2[:],
                                        in1=mterm[:], op=OP.add)
                nc.sync.dma_start(out=ml_view[:, asl], in_=ml_all[:, asl])
                nc.vector.tensor_reduce(out=rm_all[:, j:j + 1], in_=ml_all[:, asl],
                                        axis=mybir.AxisListType.X, op=OP.max)

            rm = small.tile([BL, 1], F32)
            nc.vector.tensor_reduce(out=rm[:], in_=rm_all[:],
                                    axis=mybir.AxisListType.X, op=OP.max)
            nrm = small.tile([BL, 1], F32)
            nc.vector.tensor_scalar_mul(nrm[:], rm[:], -1.0)

            s_all = small.tile([BL, ACH], F32)
            u_all = small.tile([BL, ACH], F32)
            for j in range(ACH):
                asl = slice(j * 512, (j + 1) * 512)
                ej = work.tile([BL, 512], F32, tag="ej")
                nc.scalar.activation(out=ej[:], in_=ml_all[:, asl], func=AF.Exp,
                                     bias=nrm[:, :1], accum_out=s_all[:, j:j + 1])
                ju = junkp.tile([BL, 512], F32, tag="mterm")
                nc.vector.scalar_tensor_tensor(
                    out=ju[:], in0=ej[:], scalar=1.0, in1=ml_all[:, asl],
                    op0=OP.mult, op1=OP.mult, accum_out=u_all[:, j:j + 1])

            s_t = small.tile([BL, 1], F32)
            nc.vector.tensor_reduce(out=s_t[:], in_=s_all[:],
                                    axis=mybir.AxisListType.X, op=OP.add)
            u_t = small.tile([BL, 1], F32)
            nc.vector.tensor_reduce(out=u_t[:], in_=u_all[:],
                                    axis=mybir.AxisListType.X, op=OP.add)

            logs = small.tile([BL, 1], F32)
            nc.scalar.activation(out=logs[:], in_=s_t[:], func=AF.Ln)
            lse = small.tile([BL, 1], F32)
            nc.vector.tensor_tensor(out=lse[:], in0=rm[:], in1=logs[:], op=OP.add)

            # xl = ml[b, xfers[b]] via 4B indirect gather from the round-trip
            xl = small.tile([BL, 1], F32)
            nc.gpsimd.indirect_dma_start(
                out=xl[:], out_offset=None, in_=ml_dram[:, :],
                in_offset=IndirectOffsetOnAxis(ap=xf_sb[:, :1], axis=0))
            xlp = small.tile([BL, 1], F32)
            nc.vector.tensor_tensor(out=xlp[:], in0=xl[:], in1=lse[:],
                                    op=OP.subtract)
            nc.sync.dma_start(out=xlp_o[:, None], in_=xlp[:])

            # entropy_b = logS - (U/S - rm)
            rs = small.tile([BL, 1], F32)
            nc.vector.reciprocal(out=rs[:], in_=s_t[:])
            un = small.tile([BL, 1], F32)
            nc.vector.tensor_tensor(out=un[:], in0=u_t[:], in1=rs[:],
                                    op=OP.mult)
            t2 = small.tile([BL, 1], F32)
            nc.vector.tensor_tensor(out=t2[:], in0=un[:], in1=rm[:],
                                    op=OP.subtract)
            ent = small.tile([BL, 1], F32)
            nc.vector.tensor_tensor(out=ent[:], in0=logs[:], in1=t2[:],
                                    op=OP.subtract)
            nc.sync.dma_start(out=ent_o[:, None], in_=ent[:])

    nc.compile()
    return nc


def _get_compiled():
    global _COMPILED
    if _COMPILED is None:
        _COMPILED = _build()
    return _COMPILED


def kernel(graph_embeds, next_graph_embeds, Wc1, bc1, Wc2, bc2,
           Wa1, ba1, Wa2, ba2, nodes, xfers, next_node_lists,
           is_terminals, masks):
    nc = _get_compiled()

    graph_embeds = np.ascontiguousarray(graph_embeds, dtype=np.float32)
    next_graph_embeds = np.ascontiguousarray(next_graph_embeds, dtype=np.float32)
    masks_u8 = np.ascontiguousarray(masks).astype(np.uint8)
    term_u8 = np.ascontiguousarray(is_terminals).astype(np.uint8)
    nodes = np.asarray(nodes, dtype=np.int32)
    xfers = np.asarray(xfers, dtype=np.int32)
    nnl = np.asarray(next_node_lists, dtype=np.int32)

    in_maps = []
    for c in range(NCORES):
        bs = slice(c * BL, (c + 1) * BL)
        b_loc = np.arange(BL, dtype=np.int32)
        sel_idx = b_loc * N + nodes[bs]                       # [64]
        # k-major ordering: row r = k*64 + b
        next_idx = (b_loc[None, :] * N + nnl[bs].T).reshape(-1)  # [2048]
        xf_idx = b_loc * A + xfers[bs]                        # [64]
        in_maps.append({
            "ge": graph_embeds[c * BL * N:(c + 1) * BL * N],
            "nge": next_graph_embeds[c * BL * N:(c + 1) * BL * N],
            "wc1": Wc1, "bc1": bc1,
            "wc2": np.ascontiguousarray(Wc2[:, 0]), "bc2": bc2,
            "wa1": Wa1, "ba1": ba1, "wa2": Wa2, "ba2": ba2,
            "masks": masks_u8[bs], "term": term_u8[bs],
            "sel_idx": np.ascontiguousarray(sel_idx, dtype=np.int32),
            "next_idx": np.ascontiguousarray(next_idx, dtype=np.int32),
            "xf_idx": np.ascontiguousarray(xf_idx, dtype=np.int32),
        })

    r = run_bass_kernel_spmd(nc, in_maps, core_ids=list(range(NCORES)))
    values = np.concatenate([r.results[c]["values_o"] for c in range(NCORES)])
    next_values = np.concatenate([r.results[c]["nextv_o"] for c in range(NCORES)])
    xlp = np.concatenate([r.results[c]["xlp_o"] for c in range(NCORES)])
    ent_all = np.concatenate([r.results[c]["ent_o"] for c in range(NCORES)])
    xfer_entropy = np.float32(ent_all.astype(np.float64).mean())
    return (values.astype(np.float32), next_values.astype(np.float32),
            xlp.astype(np.float32), xfer_entropy)
